# revision 12
# baseline (speedup 1.0000x reference)
"""Multi-head attention TRN2 Bass kernel, sharded over 8 NeuronCores.

Sharding: core c handles batch b = c//4 and head-group g = c%4 (4 of the 16
heads).  Each core projects its batch's q/k/v against the column slice of
Wq/Wk/Wv for its heads, runs attention for its 4 heads, and computes a
row-parallel partial of the output projection.  The host sums the 8 output
partials (adding bo) and assembles attn from per-core (h, k, q)-ordered
slices (device writes attn transposed so every DMA is contiguous; the host
transposes the last two axes while gathering).

All device matmuls run in native fp32 (exact, 4 cycles/row on the PE —
fp32r would put its ~1.2e-4 rounding directly on the stored attn).  Softmax skips the max-subtraction: logits here are
qk/8 + mask*(-1e9) with mask <= 0 elementwise, so exp never overflows and
the result is mathematically identical.

Device-side layout notes (orientation "B"):
  - qh^T, kh^T stored (depth-major): tiles (128, S) holding 2 heads each.
  - vh stored seq-major, per s-tile as (128, HG, DEPTH+1) with a trailing
    ones column so the AV matmul also accumulates the softmax denominator
    as psum row DEPTH.
  - logits^T tiles are (128 k, qc) in PSUM; ScalarE applies
    exp(logits/8 + mask_bias) straight out of PSUM (mask bias is
    per-partition = per-key, which matches orientation B).
  - the denominator reciprocal is broadcast across partitions with a K=1
    ones matmul; VectorE then normalizes attn tiles and the ctx rows.
"""

import sys

sys.path.insert(0, "/opt/trn_rl_repo")

import numpy as np

import concourse.bass as bass
import concourse.mybir as mybir
import concourse.tile as tile
from concourse.bass_utils import run_bass_kernel_spmd

from split_waits import split_multi_waits

FP = mybir.dt.float32
FPR = mybir.dt.float32r
AF = mybir.ActivationFunctionType

B = 2
S = 2048
D_MODEL = 1024
NUM_HEADS = 16
DEPTH = 64
N_CORES = 8
HG = 4  # heads per core
HD = HG * DEPTH  # 256: d_model slice per core
NEG_BIG = -1.0e9

P = 128  # partitions


def build_kernel(s=S, d_model=D_MODEL, hg=HG, depth=DEPTH, split=True):
    """Build the single-core SPMD Bass program (parameterized for sim tests).

    Per-core DRAM I/O (hd = hg*depth):
      inputs : qT,kT,vT (d_model, s); wq,wk,wv (d_model, hd); wo (hd, d_model)
               bq_col,bk_col (hd, 1); bv_row (1, hd); mask_cols (128, s/128)
      outputs: attnT (hg, s, s)  [head, key, query];  out_p (s, d_model)
    """
    hd = hg * depth
    assert s % 512 == 0 and d_model % P == 0 and hd % P == 0 and depth == 64
    n_st = s // P          # s-tiles of 128
    n_sc = s // 512        # s-chunks of 512
    n_dt = d_model // P    # d_model tiles of 128
    n_mt = hd // P         # head-dim tiles of 128 (2 heads per tile)
    QC = min(1024, s)      # query-chunk for the attention loop
    n_qc = s // QC
    nq5 = QC // 512        # 512-wide matmul slices per query chunk

    nc = bass.Bass()
    qT = nc.dram_tensor("qT", [d_model, s], FP, kind="ExternalInput")
    kT = nc.dram_tensor("kT", [d_model, s], FP, kind="ExternalInput")
    vT = nc.dram_tensor("vT", [d_model, s], FP, kind="ExternalInput")
    wq = nc.dram_tensor("wq", [d_model, hd], FP, kind="ExternalInput")
    wk = nc.dram_tensor("wk", [d_model, hd], FP, kind="ExternalInput")
    wv = nc.dram_tensor("wv", [d_model, hd], FP, kind="ExternalInput")
    wo = nc.dram_tensor("wo", [hd, d_model], FP, kind="ExternalInput")
    bq_col = nc.dram_tensor("bq_col", [hd, 1], FP, kind="ExternalInput")
    bk_col = nc.dram_tensor("bk_col", [hd, 1], FP, kind="ExternalInput")
    bv_row = nc.dram_tensor("bv_row", [1, hd], FP, kind="ExternalInput")
    mask_cols = nc.dram_tensor("mask_cols", [P, n_st], FP, kind="ExternalInput")
    attnT = nc.dram_tensor("attnT", [hg, s, s], FP, kind="ExternalOutput")
    out_p = nc.dram_tensor("out_p", [s, d_model], FP, kind="ExternalOutput")

    scale = 1.0 / np.sqrt(np.float32(depth))

    with tile.TileContext(nc) as tc:
        with tc.tile_pool(name="persist", bufs=1) as pp:
            # ---- persistent tiles ----
            qhT = [pp.tile([P, s], FP, tag=f"qhT{m}", name=f"qhT{m}") for m in range(n_mt)]
            khT = [pp.tile([P, s], FP, tag=f"khT{m}", name=f"khT{m}") for m in range(n_mt)]
            vh_aug = [
                pp.tile([P, hg, depth + 1], FP, tag=f"vh{st}", name=f"vh{st}") for st in range(n_st)
            ]
            ctx = [pp.tile([depth, s], FP, tag=f"ctx{h}", name=f"ctx{h}") for h in range(hg)]
            wo_sb = [pp.tile([depth, d_model], FP, tag=f"wo{h}", name=f"wo{h}") for h in range(hg)]
            mask_sb = pp.tile([P, n_st], FP, tag="mask")
            ones_sb = pp.tile([P, P], FP, tag="ones")

            for h in range(hg):
                nc.sync.dma_start(wo_sb[h][:], wo[h * depth : (h + 1) * depth, :])
            nc.sync.dma_start(mask_sb[:], mask_cols[:])
            nc.vector.memset(ones_sb[:], 1.0)

            # ---- projections: qh^T = wq^T @ q^T + bq, kh^T likewise ----
            with (
                tc.tile_pool(name="projw", bufs=1) as pw,
                tc.tile_pool(name="stage", bufs=3) as stage,
                tc.tile_pool(
                    name="proj_psum", bufs=2, space=bass.MemorySpace.PSUM
                ) as proj_psum,
            ):
                wq_sb = pw.tile([P, n_dt, hd], FP, tag="wq")
                wk_sb = pw.tile([P, n_dt, hd], FP, tag="wk")
                wv_sb = pw.tile([P, n_dt, hd], FP, tag="wv")
                bq_sb = [pw.tile([P, 1], FP, tag=f"bq{m}", name=f"bq{m}") for m in range(n_mt)]
                bk_sb = [pw.tile([P, 1], FP, tag=f"bk{m}", name=f"bk{m}") for m in range(n_mt)]
                bv_sb = pw.tile([1, hd], FP, tag="bv")

                nc.sync.dma_start(wq_sb[:], wq.rearrange("(dt p) m -> p dt m", p=P))
                nc.sync.dma_start(wk_sb[:], wk.rearrange("(dt p) m -> p dt m", p=P))
                nc.sync.dma_start(wv_sb[:], wv.rearrange("(dt p) m -> p dt m", p=P))
                for m in range(n_mt):
                    nc.sync.dma_start(bq_sb[m][:], bq_col[m * P : (m + 1) * P, :])
                    nc.sync.dma_start(bk_sb[m][:], bk_col[m * P : (m + 1) * P, :])
                nc.sync.dma_start(bv_sb[:], bv_row[:])
                for name, src, w_sb, b_sb, dstT in (
                    ("q", qT, wq_sb, bq_sb, qhT),
                    ("k", kT, wk_sb, bk_sb, khT),
                ):
                    for sc in range(n_sc):
                        ps = [
                            proj_psum.tile([P, 512], FP, tag=f"pp{m}", name=f"pp{m}")
                            for m in range(n_mt)
                        ]
                        for d in range(n_dt):
                            chunk = stage.tile([P, 512], FP, tag="chunk")
                            nc.sync.dma_start(
                                chunk[:],
                                src[d * P : (d + 1) * P, sc * 512 : (sc + 1) * 512],
                            )
                            for m in range(n_mt):
                                nc.tensor.matmul(
                                    ps[m][:],
                                    w_sb[:, d, m * P : (m + 1) * P],
                                    chunk[:],
                                    start=(d == 0),
                                    stop=(d == n_dt - 1),
                                )
                        for m in range(n_mt):
                            nc.vector.tensor_scalar_add(
                                dstT[m][:, sc * 512 : (sc + 1) * 512],
                                ps[m][:],
                                b_sb[m][:],
                            )

                # ---- v projection: vh = v @ wv + bv, seq-major, +ones col ----
                for st in range(n_st):
                    vchunk = stage.tile([P, n_dt, P], FP, tag="vchunk")
                    nc.sync.dma_start(
                        vchunk[:],
                        vT[:, st * P : (st + 1) * P].rearrange(
                            "(dt p) u -> p dt u", p=P
                        ),
                    )
                    pv = proj_psum.tile([P, hd], FP, tag="pv")
                    for d in range(n_dt):
                        nc.tensor.matmul(
                            pv[:],
                            vchunk[:, d, :],
                            wv_sb[:, d, :],
                            start=(d == 0),
                            stop=False,
                        )
                    nc.tensor.matmul(
                        pv[:],
                        ones_sb[0:1, :],
                        bv_sb[:],
                        start=False,
                        stop=True,
                    )
                    nc.vector.memset(vh_aug[st][:, :, depth : depth + 1], 1.0)
                    nc.vector.tensor_copy(
                        vh_aug[st][:, :, 0:depth],
                        pv[:].rearrange("p (h e) -> p h e", h=hg),
                    )

            # ---- attention ----
            with (
                tc.tile_pool(name="lg_psum", bufs=3, space=bass.MemorySpace.PSUM)
                as lg_psum,
                tc.tile_pool(name="cx_psum", bufs=1, space=bass.MemorySpace.PSUM)
                as cx_psum,
                tc.tile_pool(name="expp", bufs=n_st + 2) as expp,
                tc.tile_pool(name="attn_evac", bufs=2) as evac_pool,
            ):
                for h in range(hg):
                    mt, po = h // 2, (h % 2) * depth
                    for qc in range(n_qc):
                        q0 = qc * QC
                        pctx = cx_psum.tile([depth + 1, QC], FP, tag="pctx")
                        exps = []
                        for k in range(n_st):
                            plog = lg_psum.tile([P, QC], FP, tag="plog")
                            for j in range(nq5):
                                nc.tensor.matmul(
                                    plog[:, j * 512 : (j + 1) * 512],
                                    khT[mt][
                                        po : po + depth, k * P : (k + 1) * P
                                    ],
                                    qhT[mt][
                                        po : po + depth,
                                        q0 + j * 512 : q0 + (j + 1) * 512,
                                    ],
                                    start=True,
                                    stop=True,
                                )
                            et = expp.tile([P, QC], FP, tag="exp")
                            exps.append(et)
                            nc.scalar.activation(
                                et[:],
                                plog[:],
                                AF.Exp,
                                bias=mask_sb[:, k : k + 1],
                                scale=float(scale),
                            )
                            for j in range(nq5):
                                nc.tensor.matmul(
                                    pctx[:, j * 512 : (j + 1) * 512],
                                    vh_aug[k][:, h, :],
                                    et[:, j * 512 : (j + 1) * 512],
                                    start=(k == 0),
                                    stop=(k == n_st - 1),
                                )
                        inv = evac_pool.tile([depth + 1, QC], FP, tag="inv")
                        nc.vector.reciprocal(
                            inv[depth : depth + 1, :], pctx[depth : depth + 1, :]
                        )
                        pb = lg_psum.tile([P, QC], FP, tag="plog")
                        for j in range(nq5):
                            nc.tensor.matmul(
                                pb[:, j * 512 : (j + 1) * 512],
                                ones_sb[depth : depth + 1, :],
                                inv[
                                    depth : depth + 1, j * 512 : (j + 1) * 512
                                ],
                                start=True,
                                stop=True,
                            )
                        bcast = evac_pool.tile([P, QC], FP, tag="bcast")
                        nc.vector.tensor_copy(bcast[:], pb[:])
                        nc.vector.tensor_mul(
                            ctx[h][:, q0 : q0 + QC],
                            pctx[0:depth, :],
                            bcast[0:depth, :],
                        )
                        for k in range(n_st):
                            nc.vector.tensor_mul(exps[k][:], exps[k][:], bcast[:])
                            nc.sync.dma_start(
                                attnT[h, k * P : (k + 1) * P, q0 : q0 + QC],
                                exps[k][:],
                            )

            # ---- output projection partial: out_p = ctx_full @ wo ----
            with (
                tc.tile_pool(name="op_psum", bufs=4, space=bass.MemorySpace.PSUM)
                as op_psum,
                tc.tile_pool(name="op_out", bufs=3) as op_out,
            ):
                n5 = min(512, d_model)
                for st in range(n_st):
                    outt = op_out.tile([P, d_model], FP, tag="outt")
                    for n in range(d_model // n5):
                        po_ = op_psum.tile([P, n5], FP, tag="po")
                        for h in range(hg):
                            nc.tensor.matmul(
                                po_[:],
                                ctx[h][:, st * P : (st + 1) * P],
                                wo_sb[h][:, n * n5 : (n + 1) * n5],
                                start=(h == 0),
                                stop=(h == hg - 1),
                            )
                        nc.vector.tensor_copy(
                            outt[:, n * n5 : (n + 1) * n5], po_[:]
                        )
                    nc.sync.dma_start(out_p[st * P : (st + 1) * P, :], outt[:])

    if split:
        # CoreSim can't interpret the hoisted wait ops; walrus/HW require them.
        split_multi_waits(nc)
    return nc


_NC_CACHE = {}


def _get_nc():
    if "nc" not in _NC_CACHE:
        _NC_CACHE["nc"] = build_kernel()
    return _NC_CACHE["nc"]


def make_in_maps(v, k, q, mask, Wq, bq, Wk, bk, Wv, bv, Wo, bo):
    f32 = np.float32
    qkvT = {}
    for b in range(B):
        qkvT[b] = (
            np.ascontiguousarray(np.asarray(q[b], f32).T),
            np.ascontiguousarray(np.asarray(k[b], f32).T),
            np.ascontiguousarray(np.asarray(v[b], f32).T),
        )
    mask = np.asarray(mask, f32)
    in_maps = []
    for c in range(N_CORES):
        b, g = divmod(c, N_CORES // B)
        cols = slice(g * HD, (g + 1) * HD)
        qTb, kTb, vTb = qkvT[b]
        mb = np.ascontiguousarray(
            (mask[b, 0, 0, :] * f32(NEG_BIG)).reshape(S // P, P).T
        )
        in_maps.append(
            {
                "qT": qTb,
                "kT": kTb,
                "vT": vTb,
                "wq": np.ascontiguousarray(np.asarray(Wq, f32)[:, cols]),
                "wk": np.ascontiguousarray(np.asarray(Wk, f32)[:, cols]),
                "wv": np.ascontiguousarray(np.asarray(Wv, f32)[:, cols]),
                "wo": np.ascontiguousarray(np.asarray(Wo, f32)[cols, :]),
                "bq_col": np.ascontiguousarray(np.asarray(bq, f32)[cols, None]),
                "bk_col": np.ascontiguousarray(np.asarray(bk, f32)[cols, None]),
                "bv_row": np.ascontiguousarray(np.asarray(bv, f32)[None, cols]),
                "mask_cols": mb,
            }
        )
    return in_maps


def gather_outputs(results, bo):
    out = np.zeros((B, S, D_MODEL), np.float32)
    attn = np.empty((B, NUM_HEADS, S, S), np.float32)
    for c in range(N_CORES):
        b, g = divmod(c, N_CORES // B)
        out[b] += results[c]["out_p"]
        attn[b, g * HG : (g + 1) * HG] = results[c]["attnT"].transpose(0, 2, 1)
    out += np.asarray(bo, np.float32)
    return out, attn


def kernel(v, k, q, mask, Wq, bq, Wk, bk, Wv, bv, Wo, bo, **_unused):
    nc = _get_nc()
    in_maps = make_in_maps(v, k, q, mask, Wq, bq, Wk, bk, Wv, bv, Wo, bo)
    res = run_bass_kernel_spmd(nc, in_maps, list(range(N_CORES))).results
    return gather_outputs(res, bo)
